# revision 41
# baseline (speedup 1.0000x reference)
"""Video attention (L=2048, D=1024, 16 heads) on 8 Trainium2 NeuronCores.

Sharding: tensor-parallel over heads. Each core owns 2 heads (= 128 of the
1024 channels): Wq/Wk/Wv are split column-wise by head, Wo row-wise; each
core emits a full-shape partial output and the host sums the 8 partials
(the "all-reduce after to_out" done at gather time).

v4 changes (on top of the v2 layout; bf16 everywhere -- fp8 was
measured and rejected: with random inputs the softmax output is a
near-uniform average whose signal is ~sigma_v/sqrt(N_eff), so any
e4m3 quantization noise lands at 3-6x the 2e-2 error budget):
  - AV matmuls operand-swapped: exp-score subtiles are the stationary
    [128 keys x 128 tokens], the 65-wide ones-augmented V is the moving
    operand, so each AV matmul streams 65 rows instead of 512 (halves
    the AV stream); the softmax denominator still rides along as
    column 64.  Accumulators are [tokens, 65] slices of a pre-zeroed
    [128, 1024] PSUM tile (start=False accumulation, bank-aligned
    per-head halves).
  - normalization via per-partition reciprocal + tensor_scalar_mul
    (denominator is now a column, i.e. a per-partition scalar), then
    the y tiles are transposed back to [vchan, token] with 8 PE
    transpose instructions against a resident identity matrix.
  - phase A (projections+rope) paced in units under phase B's steps so
    the PE fills the slack while ACT chews the 64 exp tiles (the ACT
    wall, ~66us, runs within ~4us of the PE stream).
  - AV lagged 8 scores-steps behind the exp that feeds it (tapering to
    1 near the end) so the in-order PE queue never blocks on ACT.
  - single SP DMA queue in strict consumption order (the DMA pool is
    effectively serial); rope tables ship as [64, L] halves DMA'd to
    both partition halves; vaug's ones columns come from one strided
    memset on the (otherwise idle) Pool engine.
  - pipelined per-128-token tail epilogue (norm -> transpose -> y2 ->
    out-proj -> stage -> DMA per chunk) to shorten the drain.

v5/v6 loop pipelining (timing-loop variant; the single-shot program
used for correctness keeps the plain full-flush structure):
  - the hardware timing loop is 2x-unrolled with double-buffered
    resident tiles (res pool bufs=2): each body emits the NEXT body's
    input DMAs mid-stream (idx 34), hiding the ~10us DMA head ramp.
  - each body's last query-tile epilogue (transpose/out-proj/DMA) is
    carried into the next body's early steps (lands at step 2, tuned:
    earlier crowds the scores->exp ramp, later entangles the PSUM
    rings) via ping-ponged carry tiles, and each body pre-emits the
    next body's QKR(0)+QKR(1) projections + rope in its tail, so the
    in-order PE/ACT queues flow across the iteration boundary with
    ~0.5us of residual tail.  Measured dead ends kept out of the code:
    carrying V(0) or the first scores/exp pair (tail-chain
    interference), SWDGE/ACT-queue DMAs, ring-size increases, extras
    re-pacing -- each re-threads the pool rings and regressed.

v2 layout/scheduling choices (per-core):
  - everything in SBUF is bf16 (halves DMA + SBUF traffic; rel-err budget
    is 2e-2, bf16 keeps us ~1e-3); PSUM accumulation stays fp32
  - activations transposed [channel, token]; matmuls contract along
    partitions; bf16 runs 1 cycle/row at any moving size
  - V is projected *directly transposed* ([key, vchan]) by swapping the
    stationary/moving operands, killing the PE transposes of v1
  - scores [key j, query i]; softmax denominator via the ones-augmented
    65th column of V in the AV matmul (row 64 of the accumulator)
  - the (scores -> exp -> AV) chain is software-pipelined by one j-tile:
    the PE issues scores(j+1) while ACT runs exp(j), so the PE never
    stalls on the activation engine
  - output epilogue (out-proj + stage copy + DMA) for query tile i is
    deferred a few steps into tile i+1's score loop so it overlaps
  - RoPE rotate_half as a constant 128x128 sign-permutation matmul
"""

import numpy as np

import sys

sys.path.insert(0, "/opt/trn_rl_repo")

import concourse.bacc as bacc
import concourse.bass as bass
import concourse.mybir as mybir
import concourse.tile as tile

F32 = mybir.dt.float32
BF16 = mybir.dt.bfloat16
ALU = mybir.AluOpType

T, H, W, B, D = 2, 32, 32, 1, 1024
NH = 16
HD = D // NH          # 64
L = T * H * W         # 2048
NCORES = 8
C = D // NCORES       # 128 channels (2 heads) per core
NT = L // 512         # 4 token tiles of 512
KT = D // 128         # 8 contraction tiles for projections
JT = L // 128         # 16 key tiles of 128


def build_program(loop_iters=None):
    nc = bacc.Bacc("TRN2", target_bir_lowering=False, debug=False)

    # xT is host-prearranged to [128, (n k l)] so each token tile loads with
    # one straight 8KB-per-partition DMA; weights likewise [128, (k c)]
    xT = nc.dram_tensor("xT", [128, NT * KT * 512], BF16, kind="ExternalInput")
    wq = nc.dram_tensor("wq", [128, KT * C], BF16, kind="ExternalInput")
    wk = nc.dram_tensor("wk", [128, KT * C], BF16, kind="ExternalInput")
    wv = nc.dram_tensor("wv", [128, KT * C], BF16, kind="ExternalInput")
    wo = nc.dram_tensor("wo", [C, D], BF16, kind="ExternalInput")
    cosT = nc.dram_tensor("cosT", [HD, L], BF16, kind="ExternalInput")
    sinT = nc.dram_tensor("sinT", [HD, L], BF16, kind="ExternalInput")
    rmat = nc.dram_tensor("rmat", [128, 128], BF16, kind="ExternalInput")
    ident = nc.dram_tensor("ident", [128, 128], BF16, kind="ExternalInput")
    out = nc.dram_tensor("out", [L, D], BF16, kind="ExternalOutput")

    with tile.TileContext(nc) as tc:
        with (
            tc.tile_pool(name="res", bufs=1) as res,
            tc.tile_pool(name="sbw", bufs=4) as sbw,
            tc.tile_pool(name="exp", bufs=10) as expp,
            tc.tile_pool(name="ypool", bufs=2) as ypool,
            tc.tile_pool(name="scl", bufs=4) as sclp,
            tc.tile_pool(name="carry", bufs=2) as cyp,
            tc.tile_pool(name="sc", bufs=2, space="PSUM") as scp,
            tc.tile_pool(name="rotp", bufs=2, space="PSUM") as rotp,
            tc.tile_pool(name="up", bufs=1, space="PSUM") as upp,
        ):
            def emit_body():
                # ---- resident SBUF tensors; startup DMAs spread across the
                # SP/DVE/ACT HWDGE queues so descriptor generation overlaps
                wqs = res.tile([128, KT * C], BF16, tag="wq")
                wks = res.tile([128, KT * C], BF16, tag="wk")
                xts = [res.tile([128, KT * 512], BF16, tag=f"x{n}", name=f"xt{n}")
                       for n in range(NT)]
                wvs = res.tile([128, KT * C], BF16, tag="wv")
                rms = res.tile([128, 128], BF16, tag="rm")
                idt = res.tile([128, 128], BF16, tag="id")
                coss = res.tile([128, L], BF16, tag="cos")
                sins = res.tile([128, L], BF16, tag="sin")
                # v in [key, vchan] layout, 65-wide per head (65th col = 1.0
                # supplies the softmax denominator through the AV matmul)
                vaug = res.tile([128, JT * 130], BF16, tag="vaug")
                wos = res.tile([128, D], BF16, tag="wo")
                # single SP queue, strict consumption order (the DMA
                # engine pool is one serial resource; transfer order is
                # what matters, and early needs must transfer first).
                # cos/sin ship as [64, L] halves and are partition-dup'd
                # on-device; vaug's ones columns come from a strided memset.
                nc.scalar.dma_start(wqs[:], wq[:])
                nc.scalar.dma_start(xts[0][:, 0:2048], xT[:, 0:2048])
                nc.scalar.dma_start(wks[:], wk[:])
                nc.scalar.dma_start(xts[0][:, 2048:4096], xT[:, 2048:4096])
                nc.sync.dma_start(rms[:], rmat[:])
                nc.sync.dma_start(coss[0:64, :], cosT[:])
                nc.sync.dma_start(coss[64:128, :], cosT[:])
                nc.sync.dma_start(sins[0:64, :], sinT[:])
                nc.sync.dma_start(sins[64:128, :], sinT[:])
                nc.sync.dma_start(wvs[:], wv[:])
                nc.sync.dma_start(xts[1][:, 0:2048], xT[:, 4096:6144])
                nc.sync.dma_start(xts[1][:, 2048:4096], xT[:, 6144:8192])
                nc.sync.dma_start(xts[2][:], xT[:, 8192:12288])
                nc.sync.dma_start(xts[3][:], xT[:, 12288:16384])
                nc.sync.dma_start(idt[:], ident[:])
                nc.sync.dma_start(wos[:], wo[:])
                nc.gpsimd.memset(
                    vaug[:].rearrange("p (b c) -> p b c", c=65)[:, :, 64:65],
                    1.0)

                qns = res.tile([128, L], BF16, tag="qn")
                kns = res.tile([128, L], BF16, tag="kn")

                # warm the ACT exp table while DMAs are in flight
                warm = sbw.tile([1, 2], F32, tag="warm")
                nc.vector.memset(warm[:], 0.0)
                warm2 = sbw.tile([1, 2], BF16, tag="warm2")
                nc.scalar.activation(warm2[:], warm[:],
                                     mybir.ActivationFunctionType.Exp)

                # ---- phase A units: QK projection + rope for one 512-token
                # tile, and direct-transposed V in 2-keytile halves
                def emit_qkr(n):
                    nsl = slice(512 * n, 512 * (n + 1))
                    xt = xts[n]
                    ps = scp.tile([128, 1024], F32, tag="sc", name=f"pqk{n}")
                    for kk_blk in (range(0, 4), range(4, KT)):
                        for wsb, half in ((wqs, 0), (wks, 1)):
                            for kk in kk_blk:
                                nc.tensor.matmul(
                                    ps[:, 512 * half:512 * (half + 1)],
                                    wsb[:, C * kk:C * (kk + 1)],
                                    xt[:, 512 * kk:512 * (kk + 1)],
                                    start=(kk == 0),
                                    stop=(kk == KT - 1),
                                )
                    cq = sbw.tile([128, 512], BF16, tag="cq")
                    ck = sbw.tile([128, 512], BF16, tag="ck")
                    nc.scalar.copy(cq[:], ps[:, 0:512])
                    nc.vector.tensor_copy(ck[:], ps[:, 512:1024])
                    for craw, dest in ((cq, qns), (ck, kns)):
                        rot = rotp.tile([128, 512], F32, tag="rot")
                        nc.tensor.matmul(rot[:], rms[:], craw[:], start=True,
                                         stop=True)
                        qc = sbw.tile([128, 512], BF16, tag="qc")
                        nc.vector.tensor_mul(qc[:], craw[:], coss[:, nsl])
                        t2 = sbw.tile([128, 512], BF16, tag="t2")
                        nc.vector.tensor_mul(t2[:], rot[:], sins[:, nsl])
                        nc.vector.tensor_add(dest[:, nsl], qc[:], t2[:])

                def emit_v(n, half):
                    xt = xts[n]
                    vps = rotp.tile([128, 512], F32, tag="rot",
                                    name=f"v{n}_{half}")
                    for jj in (2 * half, 2 * half + 1):
                        for kk in range(KT):
                            nc.tensor.matmul(
                                vps[:, 128 * (jj - 2 * half):
                                    128 * (jj - 2 * half + 1)],
                                xt[:, 512 * kk + 128 * jj:
                                   512 * kk + 128 * (jj + 1)],
                                wvs[:, C * kk:C * (kk + 1)],
                                start=(kk == 0),
                                stop=(kk == KT - 1),
                            )
                    for jj in (2 * half, 2 * half + 1):
                        j = 4 * n + jj
                        jl = jj - 2 * half
                        nc.vector.tensor_copy(
                            vaug[:, 130 * j:130 * j + 64],
                            vps[:, 128 * jl:128 * jl + 64])
                        nc.vector.tensor_copy(
                            vaug[:, 130 * j + 65:130 * j + 129],
                            vps[:, 128 * jl + 64:128 * jl + 128])

                # ---- phase B: attention, AV lagged 4 scores-steps behind
                u = {}
                exps = {}
                av_started = {}

                def emit_scores(i, j):
                    isl = slice(512 * i, 512 * (i + 1))
                    sps = scp.tile([128, 1024], F32, tag="sc", name=f"s{i}_{j}")
                    for h in range(2):
                        hp = slice(64 * h, 64 * (h + 1))
                        nc.tensor.matmul(
                            sps[:, 512 * h:512 * (h + 1)],
                            kns[hp, 128 * j:128 * (j + 1)],
                            qns[hp, isl],
                            start=True,
                            stop=True,
                        )
                    e = expp.tile([128, 1024], BF16, tag="e")
                    nc.scalar.activation(e[:], sps[:], mybir.ActivationFunctionType.Exp)
                    exps[(i, j)] = e

                def emit_av(i, j, n_done):
                    # swapped operands: e-subtiles stationary, vaug moving
                    # (F=65); out [tokens, 64 vchan | den] per (h, m) slice.
                    # u is pre-zeroed by Pool memset; all matmuls accumulate.
                    e = exps.pop((i, j))
                    ut = u[i]
                    for h in range(2):
                        vm = vaug[:, 130 * j + 65 * h:130 * j + 65 * (h + 1)]
                        for m in range(4):
                            off = 512 * h + 65 * m
                            nc.tensor.matmul(
                                ut[:, off:off + 65],
                                e[:, 512 * h + 128 * m:512 * h + 128 * (m + 1)],
                                vm,
                                start=False,
                                stop=False,
                                skip_group_check=True,
                            )

                def emit_norm(i):
                    # y-sub[tokens, vchan] = u / den (den = col 64 per slice)
                    ut = u[i]
                    rec8 = sclp.tile([128, 8], F32, tag="rec")
                    for h in range(2):
                        nc.vector.reciprocal(
                            rec8[:, 4 * h:4 * (h + 1)].rearrange(
                                "p (m c) -> p m c", c=1),
                            ut[:, 512 * h:512 * h + 260].rearrange(
                                "p (m c) -> p m c", m=4)[:, :, 64:65])
                    ys = ypool.tile([128, 512], BF16, tag="ys", name=f"ys{i}")
                    for h in range(2):
                        for m in range(4):
                            g = 4 * h + m
                            nc.vector.tensor_scalar_mul(
                                ys[:, 64 * g:64 * (g + 1)],
                                ut[:, 512 * h + 65 * m:512 * h + 65 * m + 64],
                                rec8[:, g:g + 1])
                    del u[i]
                    return ys

                def emit_transpose(i, ys):
                    # ys slices [128 tokens, 64 vchan] -> y2 [128 vchan, 512]
                    tp = rotp.tile([128, 1024], BF16, tag="rot", name=f"tp{i}")
                    for h in range(2):
                        for m in range(4):
                            nc.tensor.matmul(
                                tp[64 * h:64 * (h + 1), 128 * m:128 * (m + 1)],
                                ys[:, 64 * (4 * h + m):64 * (4 * h + m + 1)],
                                idt[:],
                                is_transpose=True,
                            )
                    y2 = ypool.tile([128, 512], BF16, tag="y", name=f"y{i}")
                    nc.vector.tensor_copy(y2[:], tp[:, 0:512])
                    return y2

                def emit_tail_epilogue(i):
                    # per-128-token-chunk chain: norm -> transpose -> y2 copy
                    # -> outproj -> stage -> DMA, so the stages pipeline
                    ut = u[i]
                    rec8 = sclp.tile([128, 8], F32, tag="rec")
                    for h in range(2):
                        nc.vector.reciprocal(
                            rec8[:, 4 * h:4 * (h + 1)].rearrange(
                                "p (m c) -> p m c", c=1),
                            ut[:, 512 * h:512 * h + 260].rearrange(
                                "p (m c) -> p m c", m=4)[:, :, 64:65])
                    ys = ypool.tile([128, 512], BF16, tag="ys", name=f"ys{i}")
                    tp = rotp.tile([128, 1024], BF16, tag="rot", name=f"tp{i}")
                    y2 = ypool.tile([128, 512], BF16, tag="y", name=f"y{i}")
                    stage = ypool.tile([128, 4096], BF16, tag="stage",
                                       name=f"st{i}")
                    for m in range(4):
                        for h in range(2):
                            g = 4 * h + m
                            nc.vector.tensor_scalar_mul(
                                ys[:, 64 * g:64 * (g + 1)],
                                ut[:, 512 * h + 65 * m:512 * h + 65 * m + 64],
                                rec8[:, g:g + 1])
                            nc.tensor.matmul(
                                tp[64 * h:64 * (h + 1), 128 * m:128 * (m + 1)],
                                ys[:, 64 * g:64 * (g + 1)],
                                idt[:],
                                is_transpose=True,
                            )
                        nc.scalar.copy(y2[:, 128 * m:128 * (m + 1)],
                                       tp[:, 128 * m:128 * (m + 1)])
                        ops_ = scp.tile([128, 1024], F32, tag="sc",
                                        name=f"opt{i}_{m}")
                        for n2 in range(2):
                            nc.tensor.matmul(
                                ops_[:, 512 * n2:512 * (n2 + 1)],
                                y2[:, 128 * m:128 * (m + 1)],
                                wos[:, 512 * n2:512 * (n2 + 1)],
                                start=True,
                                stop=True,
                            )
                        dst = stage[:, 1024 * m:1024 * (m + 1)]
                        if m % 2 == 0:
                            nc.vector.tensor_copy(dst, ops_[:])
                        else:
                            nc.scalar.copy(dst, ops_[:])
                        nc.sync.dma_start(
                            out[512 * i + 128 * m:512 * i + 128 * (m + 1), :],
                            dst,
                        )
                    del u[i]

                def emit_outproj(i, y, tail=False):
                    stage = ypool.tile([128, 4096], BF16, tag="stage", name=f"st{i}")
                    for m in range(4):
                        if tail:
                            # scores PSUM banks are free in the tail: pair the
                            # two 512-halves in one tile, one big stage copy
                            ops_ = scp.tile([128, 1024], F32, tag="sc",
                                            name=f"opt{i}_{m}")
                            for n2 in range(2):
                                nc.tensor.matmul(
                                    ops_[:, 512 * n2:512 * (n2 + 1)],
                                    y[:, 128 * m:128 * (m + 1)],
                                    wos[:, 512 * n2:512 * (n2 + 1)],
                                    start=True,
                                    stop=True,
                                )
                            dst = stage[:, 1024 * m:1024 * (m + 1)]
                            if m % 2 == 0:
                                nc.vector.tensor_copy(dst, ops_[:])
                            else:
                                nc.scalar.copy(dst, ops_[:])
                        else:
                            for n2 in range(2):
                                ops_ = rotp.tile([128, 512], F32, tag="rot")
                                nc.tensor.matmul(
                                    ops_[:],
                                    y[:, 128 * m:128 * (m + 1)],
                                    wos[:, 512 * n2:512 * (n2 + 1)],
                                    start=True,
                                    stop=True,
                                )
                                dst = stage[:, 1024 * m + 512 * n2:
                                            1024 * m + 512 * (n2 + 1)]
                                nc.vector.tensor_copy(dst, ops_[:])
                        # fire the DMA for this 128-token chunk immediately
                        nc.sync.dma_start(
                            out[512 * i + 128 * m:512 * i + 128 * (m + 1), :],
                            stage[:, 1024 * m:1024 * (m + 1)],
                        )

                # ---- schedule: A units paced under the B steps; AV lag
                # tapers from 8 (ACT queue deep early) to 1 (short tail)
                def av_lag(at):
                    return 6 if at < 44 else (4 if at < 56 else 1)

                extras = {
                    1: [("QKR", 1)], 2: [("V", 0, 0)], 3: [("V", 0, 1)],
                    4: [("QKR", 2)], 6: [("QKR", 3)], 8: [("V", 1, 0)],
                    10: [("V", 1, 1)], 12: [("V", 2, 0)], 14: [("V", 2, 1)],
                    16: [("V", 3, 0)], 18: [("V", 3, 1)],
                }
                emit_qkr(0)

                ys = {}
                av_q = []       # (i, j, idx when exp emitted)
                pending = []    # deferred events: (at_idx, kind, i)
                av_count = {}
                idx = 0
                for i in range(NT):
                    u[i] = upp.tile([128, 1024], F32, tag="u", name=f"u{i}")
                    nc.vector.memset(u[i][:, 0:260], 0.0)
                    nc.vector.memset(u[i][:, 512:772], 0.0)
                    av_count[i] = 0
                    for j in range(JT):
                        while av_q and av_q[0][2] + av_lag(idx) <= idx:
                            ai, aj, _ = av_q.pop(0)
                            emit_av(ai, aj, av_count[ai])
                            av_count[ai] += 1
                            if av_count[ai] == JT:
                                ys[ai] = emit_norm(ai)
                                pending.append((idx + 1, "tr", ai))
                        emit_scores(i, j)
                        av_q.append((i, j, idx))
                        for unit in extras.get(idx, ()):
                            if unit[0] == "QKR":
                                emit_qkr(unit[1])
                            else:
                                emit_v(unit[1], unit[2])
                        while pending and pending[0][0] <= idx:
                            _, kind, ei = pending.pop(0)
                            if kind == "tr":
                                ys[ei] = emit_transpose(ei, ys.pop(ei))
                                pending.append((idx + 1, "op", ei))
                            else:
                                emit_outproj(ei, ys.pop(ei))
                        idx += 1
                done_i = []
                for ai, aj, _ in av_q:
                    emit_av(ai, aj, av_count[ai])
                    av_count[ai] += 1
                    if av_count[ai] == JT:
                        done_i.append(ai)
                for _, kind, ei in sorted(pending):
                    if kind == "tr":
                        ys[ei] = emit_transpose(ei, ys.pop(ei))
                        emit_outproj(ei, ys.pop(ei), tail=True)
                    else:
                        emit_outproj(ei, ys.pop(ei), tail=True)
                for ai in done_i:
                    emit_tail_epilogue(ai)

            if loop_iters is None:
                emit_body()
            else:
                with tc.For_i(0, loop_iters, 1):
                    emit_body()

    nc.compile()
    return nc


_NC = None


def _get_nc():
    global _NC
    if _NC is None:
        _NC = build_program()
    return _NC


def make_in_maps(x, rope_emb_L_1_1_D, Wq, Wk, Wv, Wo):
    """Host-side prep: shard weights by head, transpose x, build rope tables.
    Everything shipped to the device as bf16."""
    import ml_dtypes

    bf16 = ml_dtypes.bfloat16
    x = np.asarray(x, dtype=np.float32)
    rope = np.asarray(rope_emb_L_1_1_D, dtype=np.float32).reshape(L, HD)
    Wq = np.asarray(Wq, dtype=np.float32)
    Wk = np.asarray(Wk, dtype=np.float32)
    Wv = np.asarray(Wv, dtype=np.float32)
    Wo = np.asarray(Wo, dtype=np.float32)

    xs_flat = x.reshape(L, D)  # B == 1
    # [128, (n k l)]: element (p, n, k, l) = x[512n+l, 128k+p]
    xT = np.ascontiguousarray(
        xs_flat.reshape(NT, 512, KT, 128).transpose(3, 0, 2, 1).reshape(
            128, NT * KT * 512)).astype(bf16)

    def wprep(wt):  # [D, C] -> [128, (k c)]: (p, k, c) = wt[128k+p, c]
        return np.ascontiguousarray(
            wt.reshape(KT, 128, C).transpose(1, 0, 2).reshape(128, KT * C)
        ).astype(bf16)

    cosT = np.cos(rope).T.astype(bf16)  # [HD, L]
    sinT = np.sin(rope).T.astype(bf16)

    # rot(q)[d'] = sum_k rmat[k, d'] q[k]; per 64-block: first 32 rows get
    # -q[d+32], last 32 get +q[d-32]  (signs folded in so sinT is plain sin)
    rmat = np.zeros((128, 128), dtype=np.float32)
    for b in (0, 64):
        for m in range(32):
            rmat[b + m + 32, b + m] = -1.0
        for m in range(32, 64):
            rmat[b + m - 32, b + m] = 1.0
    rmat = rmat.astype(bf16)

    scale = HD ** -0.5
    identm = np.eye(128, dtype=np.float32).astype(bf16)
    in_maps = []
    for c in range(NCORES):
        rows = slice(C * c, C * (c + 1))
        in_maps.append({
            "xT": xT,
            "wq": wprep((scale * Wq[rows, :]).T),
            "wk": wprep(Wk[rows, :].T),
            "wv": wprep(Wv[rows, :].T),
            "wo": np.ascontiguousarray(Wo[:, rows].T).astype(bf16),
            "cosT": cosT,
            "sinT": sinT,
            "rmat": rmat,
            "ident": identm,
        })
    return in_maps


class _Runner:
    """Persistent jitted SPMD executable (mirrors bass2jax.run_bass_via_pjrt
    but caches the compiled callable, and builds the donated output buffers
    on-device instead of shipping zeros through the tunnel)."""

    def __init__(self, nc):
        import jax
        import jax.numpy as jnp
        from jax.sharding import Mesh, PartitionSpec, NamedSharding
        from jax.experimental.shard_map import shard_map
        from concourse import bass2jax

        bass2jax.install_neuronx_cc_hook()
        self.jax = jax
        self.nc = nc
        part_name = nc.partition_id_tensor.name if nc.partition_id_tensor else None
        in_names, out_names, out_avals, zero_shapes = [], [], [], []
        for alloc in nc.m.functions[0].allocations:
            if not isinstance(alloc, mybir.MemoryLocationSet):
                continue
            name = alloc.memorylocations[0].name
            if alloc.kind == "ExternalInput":
                if name != part_name:
                    in_names.append(name)
            elif alloc.kind == "ExternalOutput":
                out_names.append(name)
                shape = tuple(alloc.tensor_shape)
                dtype = mybir.dt.np(alloc.dtype)
                out_avals.append(jax.core.ShapedArray(shape, dtype))
                zero_shapes.append((shape, dtype))
        self.in_names = list(in_names)
        self.out_names = list(out_names)
        self.out_avals = out_avals
        self.zero_shapes = zero_shapes
        n_params = len(in_names)
        n_outs = len(out_names)
        all_in_names = in_names + out_names
        if part_name is not None:
            all_in_names = all_in_names + [part_name]

        def _body(*args):
            operands = list(args)
            if part_name is not None:
                operands.append(bass2jax.partition_id_tensor())
            outs = bass2jax._bass_exec_p.bind(
                *operands,
                out_avals=tuple(out_avals),
                in_names=tuple(all_in_names),
                out_names=tuple(out_names),
                lowering_input_output_aliases=(),
                sim_require_finite=True,
                sim_require_nnan=True,
                nc=nc,
            )
            return tuple(outs)

        devices = jax.devices()[:NCORES]
        self.mesh = Mesh(np.asarray(devices), ("core",))
        self.pspec = PartitionSpec("core")
        self.sh = NamedSharding(self.mesh, self.pspec)
        in_specs = (self.pspec,) * (n_params + n_outs)
        out_specs = (self.pspec,) * n_outs
        self.sharded = jax.jit(
            shard_map(_body, mesh=self.mesh, in_specs=in_specs,
                      out_specs=out_specs, check_rep=False),
            donate_argnums=tuple(range(n_params, n_params + n_outs)),
            keep_unused=True,
        )
        # donated output buffers built on-device (fresh ones per call)
        self._zeros_fn = jax.jit(
            lambda: tuple(
                jnp.zeros((NCORES * s[0], *s[1:]), dt) for s, dt in zero_shapes
            ),
            out_shardings=tuple(self.sh for _ in zero_shapes),
        )

    def concat_inputs(self, in_maps):
        return [
            np.concatenate([np.asarray(m[name]) for m in in_maps], axis=0)
            for name in self.in_names
        ]

    def device_inputs(self, in_maps):
        return [self.jax.device_put(a, self.sh) for a in self.concat_inputs(in_maps)]

    def fresh_zeros(self):
        return list(self._zeros_fn())

    def __call__(self, dev_in, zeros):
        outs = self.sharded(*dev_in, *zeros)
        self.jax.block_until_ready(outs)
        return outs

    def run_np(self, in_maps):
        outs = self(self.device_inputs(in_maps), self.fresh_zeros())
        per_core = []
        for c in range(NCORES):
            d = {}
            for idx, name in enumerate(self.out_names):
                shape = self.out_avals[idx].shape
                d[name] = np.asarray(outs[idx]).reshape(NCORES, *shape)[c]
            per_core.append(d)
        return per_core


_RUNNER = None


def _get_runner():
    global _RUNNER
    if _RUNNER is None:
        _RUNNER = _Runner(_get_nc())
    return _RUNNER


def run(inputs):
    runner = _get_runner()
    in_maps = make_in_maps(**inputs)
    results = runner.run_np(in_maps)
    partial = np.zeros((L, D), dtype=np.float32)
    for r in results:
        partial += r["out"].astype(np.float32)
    return partial.reshape(T, H, W, B, D)


def kernel(**inputs):
    return run(inputs)



# revision 43
# speedup vs baseline: 1.1384x; 1.1384x over previous
"""Video attention (L=2048, D=1024, 16 heads) on 8 Trainium2 NeuronCores.

Sharding: tensor-parallel over heads. Each core owns 2 heads (= 128 of the
1024 channels): Wq/Wk/Wv are split column-wise by head, Wo row-wise; each
core emits a full-shape partial output and the host sums the 8 partials
(the "all-reduce after to_out" done at gather time).

v4 changes (on top of the v2 layout; bf16 everywhere -- fp8 was
measured and rejected: with random inputs the softmax output is a
near-uniform average whose signal is ~sigma_v/sqrt(N_eff), so any
e4m3 quantization noise lands at 3-6x the 2e-2 error budget):
  - AV matmuls operand-swapped: exp-score subtiles are the stationary
    [128 keys x 128 tokens], the 65-wide ones-augmented V is the moving
    operand, so each AV matmul streams 65 rows instead of 512 (halves
    the AV stream); the softmax denominator still rides along as
    column 64.  Accumulators are [tokens, 65] slices of a pre-zeroed
    [128, 1024] PSUM tile (start=False accumulation, bank-aligned
    per-head halves).
  - normalization via per-partition reciprocal + tensor_scalar_mul
    (denominator is now a column, i.e. a per-partition scalar), then
    the y tiles are transposed back to [vchan, token] with 8 PE
    transpose instructions against a resident identity matrix.
  - phase A (projections+rope) paced in units under phase B's steps so
    the PE fills the slack while ACT chews the 64 exp tiles (the ACT
    wall, ~66us, runs within ~4us of the PE stream).
  - AV lagged 8 scores-steps behind the exp that feeds it (tapering to
    1 near the end) so the in-order PE queue never blocks on ACT.
  - single SP DMA queue in strict consumption order (the DMA pool is
    effectively serial); rope tables ship as [64, L] halves DMA'd to
    both partition halves; vaug's ones columns come from one strided
    memset on the (otherwise idle) Pool engine.
  - pipelined per-128-token tail epilogue (norm -> transpose -> y2 ->
    out-proj -> stage -> DMA per chunk) to shorten the drain.

v5/v6 loop pipelining (timing-loop variant; the single-shot program
used for correctness keeps the plain full-flush structure):
  - the hardware timing loop is 2x-unrolled with double-buffered
    resident tiles (res pool bufs=2): each body emits the NEXT body's
    input DMAs mid-stream (idx 34), hiding the ~10us DMA head ramp.
  - each body's last query-tile epilogue (transpose/out-proj/DMA) is
    carried into the next body's early steps (lands at step 2, tuned:
    earlier crowds the scores->exp ramp, later entangles the PSUM
    rings) via ping-ponged carry tiles, and each body pre-emits the
    next body's QKR(0)+QKR(1) projections + rope in its tail, so the
    in-order PE/ACT queues flow across the iteration boundary with
    ~0.5us of residual tail.  Measured dead ends kept out of the code:
    carrying V(0) or the first scores/exp pair (tail-chain
    interference), SWDGE/ACT-queue DMAs, ring-size increases, extras
    re-pacing -- each re-threads the pool rings and regressed.

v2 layout/scheduling choices (per-core):
  - everything in SBUF is bf16 (halves DMA + SBUF traffic; rel-err budget
    is 2e-2, bf16 keeps us ~1e-3); PSUM accumulation stays fp32
  - activations transposed [channel, token]; matmuls contract along
    partitions; bf16 runs 1 cycle/row at any moving size
  - V is projected *directly transposed* ([key, vchan]) by swapping the
    stationary/moving operands, killing the PE transposes of v1
  - scores [key j, query i]; softmax denominator via the ones-augmented
    65th column of V in the AV matmul (row 64 of the accumulator)
  - the (scores -> exp -> AV) chain is software-pipelined by one j-tile:
    the PE issues scores(j+1) while ACT runs exp(j), so the PE never
    stalls on the activation engine
  - output epilogue (out-proj + stage copy + DMA) for query tile i is
    deferred a few steps into tile i+1's score loop so it overlaps
  - RoPE rotate_half as a constant 128x128 sign-permutation matmul
"""

import numpy as np

import sys

sys.path.insert(0, "/opt/trn_rl_repo")

import concourse.bacc as bacc
import concourse.bass as bass
import concourse.mybir as mybir
import concourse.tile as tile

F32 = mybir.dt.float32
BF16 = mybir.dt.bfloat16
ALU = mybir.AluOpType

T, H, W, B, D = 2, 32, 32, 1, 1024
NH = 16
HD = D // NH          # 64
L = T * H * W         # 2048
NCORES = 8
C = D // NCORES       # 128 channels (2 heads) per core
NT = L // 512         # 4 token tiles of 512
KT = D // 128         # 8 contraction tiles for projections
JT = L // 128         # 16 key tiles of 128


def build_program(loop_iters=None):
    nc = bacc.Bacc("TRN2", target_bir_lowering=False, debug=False)

    # xT is host-prearranged to [128, (n k l)] so each token tile loads with
    # one straight 8KB-per-partition DMA; weights likewise [128, (k c)]
    xT = nc.dram_tensor("xT", [128, NT * KT * 512], BF16, kind="ExternalInput")
    wq = nc.dram_tensor("wq", [128, KT * C], BF16, kind="ExternalInput")
    wk = nc.dram_tensor("wk", [128, KT * C], BF16, kind="ExternalInput")
    wv = nc.dram_tensor("wv", [128, KT * C], BF16, kind="ExternalInput")
    wo = nc.dram_tensor("wo", [C, D], BF16, kind="ExternalInput")
    cosT = nc.dram_tensor("cosT", [HD, L], BF16, kind="ExternalInput")
    sinT = nc.dram_tensor("sinT", [HD, L], BF16, kind="ExternalInput")
    rmat = nc.dram_tensor("rmat", [128, 128], BF16, kind="ExternalInput")
    ident = nc.dram_tensor("ident", [128, 128], BF16, kind="ExternalInput")
    out = nc.dram_tensor("out", [L, D], BF16, kind="ExternalOutput")

    with tile.TileContext(nc) as tc:
        with (
            tc.tile_pool(name="res", bufs=1) as res,
            tc.tile_pool(name="sbw", bufs=4) as sbw,
            tc.tile_pool(name="exp", bufs=13) as expp,
            tc.tile_pool(name="ypool", bufs=2) as ypool,
            tc.tile_pool(name="scl", bufs=4) as sclp,
            tc.tile_pool(name="carry", bufs=2) as cyp,
            tc.tile_pool(name="sc", bufs=2, space="PSUM") as scp,
            tc.tile_pool(name="rotp", bufs=2, space="PSUM") as rotp,
            tc.tile_pool(name="up", bufs=1, space="PSUM") as upp,
        ):
            def emit_body():
                # ---- resident SBUF tensors; startup DMAs spread across the
                # SP/DVE/ACT HWDGE queues so descriptor generation overlaps
                wqs = res.tile([128, KT * C], BF16, tag="wq")
                wks = res.tile([128, KT * C], BF16, tag="wk")
                xts = [res.tile([128, KT * 512], BF16, tag=f"x{n}", name=f"xt{n}")
                       for n in range(NT)]
                wvs = res.tile([128, KT * C], BF16, tag="wv")
                rms = res.tile([128, 128], BF16, tag="rm")
                idt = res.tile([128, 128], BF16, tag="id")
                coss = res.tile([128, L], BF16, tag="cos")
                sins = res.tile([128, L], BF16, tag="sin")
                # v in [key, vchan] layout, 65-wide per head (65th col = 1.0
                # supplies the softmax denominator through the AV matmul)
                vaug = res.tile([128, JT * 130], BF16, tag="vaug")
                wos = res.tile([128, D], BF16, tag="wo")
                # single SP queue, strict consumption order (the DMA
                # engine pool is one serial resource; transfer order is
                # what matters, and early needs must transfer first).
                # cos/sin ship as [64, L] halves and are partition-dup'd
                # on-device; vaug's ones columns come from a strided memset.
                nc.scalar.dma_start(wqs[:], wq[:])
                nc.scalar.dma_start(xts[0][:, 0:2048], xT[:, 0:2048])
                nc.scalar.dma_start(wks[:], wk[:])
                nc.scalar.dma_start(xts[0][:, 2048:4096], xT[:, 2048:4096])
                nc.sync.dma_start(rms[:], rmat[:])
                nc.sync.dma_start(coss[0:64, :], cosT[:])
                nc.sync.dma_start(coss[64:128, :], cosT[:])
                nc.sync.dma_start(sins[0:64, :], sinT[:])
                nc.sync.dma_start(sins[64:128, :], sinT[:])
                nc.sync.dma_start(wvs[:], wv[:])
                nc.sync.dma_start(xts[1][:, 0:2048], xT[:, 4096:6144])
                nc.sync.dma_start(xts[1][:, 2048:4096], xT[:, 6144:8192])
                nc.sync.dma_start(xts[2][:], xT[:, 8192:12288])
                nc.sync.dma_start(xts[3][:], xT[:, 12288:16384])
                nc.sync.dma_start(idt[:], ident[:])
                nc.sync.dma_start(wos[:], wo[:])
                nc.gpsimd.memset(
                    vaug[:].rearrange("p (b c) -> p b c", c=65)[:, :, 64:65],
                    1.0)

                qns = res.tile([128, L], BF16, tag="qn")
                kns = res.tile([128, L], BF16, tag="kn")

                # warm the ACT exp table while DMAs are in flight
                warm = sbw.tile([1, 2], F32, tag="warm")
                nc.vector.memset(warm[:], 0.0)
                warm2 = sbw.tile([1, 2], BF16, tag="warm2")
                nc.scalar.activation(warm2[:], warm[:],
                                     mybir.ActivationFunctionType.Exp)

                # ---- phase A units: QK projection + rope for one 512-token
                # tile, and direct-transposed V in 2-keytile halves
                def emit_qkr(n):
                    nsl = slice(512 * n, 512 * (n + 1))
                    xt = xts[n]
                    ps = scp.tile([128, 1024], F32, tag="sc", name=f"pqk{n}")
                    for kk_blk in (range(0, 4), range(4, KT)):
                        for wsb, half in ((wqs, 0), (wks, 1)):
                            for kk in kk_blk:
                                nc.tensor.matmul(
                                    ps[:, 512 * half:512 * (half + 1)],
                                    wsb[:, C * kk:C * (kk + 1)],
                                    xt[:, 512 * kk:512 * (kk + 1)],
                                    start=(kk == 0),
                                    stop=(kk == KT - 1),
                                )
                    cq = sbw.tile([128, 512], BF16, tag="cq")
                    ck = sbw.tile([128, 512], BF16, tag="ck")
                    nc.scalar.copy(cq[:], ps[:, 0:512])
                    nc.vector.tensor_copy(ck[:], ps[:, 512:1024])
                    for craw, dest in ((cq, qns), (ck, kns)):
                        rot = rotp.tile([128, 512], F32, tag="rot")
                        nc.tensor.matmul(rot[:], rms[:], craw[:], start=True,
                                         stop=True)
                        qc = sbw.tile([128, 512], BF16, tag="qc")
                        nc.vector.tensor_mul(qc[:], craw[:], coss[:, nsl])
                        t2 = sbw.tile([128, 512], BF16, tag="t2")
                        nc.vector.tensor_mul(t2[:], rot[:], sins[:, nsl])
                        nc.vector.tensor_add(dest[:, nsl], qc[:], t2[:])

                def emit_v(n, half):
                    xt = xts[n]
                    vps = rotp.tile([128, 512], F32, tag="rot",
                                    name=f"v{n}_{half}")
                    for jj in (2 * half, 2 * half + 1):
                        for kk in range(KT):
                            nc.tensor.matmul(
                                vps[:, 128 * (jj - 2 * half):
                                    128 * (jj - 2 * half + 1)],
                                xt[:, 512 * kk + 128 * jj:
                                   512 * kk + 128 * (jj + 1)],
                                wvs[:, C * kk:C * (kk + 1)],
                                start=(kk == 0),
                                stop=(kk == KT - 1),
                            )
                    for jj in (2 * half, 2 * half + 1):
                        j = 4 * n + jj
                        jl = jj - 2 * half
                        nc.vector.tensor_copy(
                            vaug[:, 130 * j:130 * j + 64],
                            vps[:, 128 * jl:128 * jl + 64])
                        nc.vector.tensor_copy(
                            vaug[:, 130 * j + 65:130 * j + 129],
                            vps[:, 128 * jl + 64:128 * jl + 128])

                # ---- phase B: attention, AV lagged 4 scores-steps behind
                u = {}
                exps = {}
                av_started = {}

                def emit_scores(i, j):
                    isl = slice(512 * i, 512 * (i + 1))
                    sps = scp.tile([128, 1024], F32, tag="sc", name=f"s{i}_{j}")
                    for h in range(2):
                        hp = slice(64 * h, 64 * (h + 1))
                        nc.tensor.matmul(
                            sps[:, 512 * h:512 * (h + 1)],
                            kns[hp, 128 * j:128 * (j + 1)],
                            qns[hp, isl],
                            start=True,
                            stop=True,
                        )
                    e = expp.tile([128, 1024], BF16, tag="e")
                    nc.scalar.activation(e[:], sps[:], mybir.ActivationFunctionType.Exp)
                    exps[(i, j)] = e

                def emit_av(i, j, n_done):
                    # swapped operands: e-subtiles stationary, vaug moving
                    # (F=65); out [tokens, 64 vchan | den] per (h, m) slice.
                    # u is pre-zeroed by Pool memset; all matmuls accumulate.
                    e = exps.pop((i, j))
                    ut = u[i]
                    for h in range(2):
                        vm = vaug[:, 130 * j + 65 * h:130 * j + 65 * (h + 1)]
                        for m in range(4):
                            off = 512 * h + 65 * m
                            nc.tensor.matmul(
                                ut[:, off:off + 65],
                                e[:, 512 * h + 128 * m:512 * h + 128 * (m + 1)],
                                vm,
                                start=False,
                                stop=False,
                                skip_group_check=True,
                            )

                def emit_norm(i):
                    # y-sub[tokens, vchan] = u / den (den = col 64 per slice)
                    ut = u[i]
                    rec8 = sclp.tile([128, 8], F32, tag="rec")
                    for h in range(2):
                        nc.vector.reciprocal(
                            rec8[:, 4 * h:4 * (h + 1)].rearrange(
                                "p (m c) -> p m c", c=1),
                            ut[:, 512 * h:512 * h + 260].rearrange(
                                "p (m c) -> p m c", m=4)[:, :, 64:65])
                    ys = ypool.tile([128, 512], BF16, tag="ys", name=f"ys{i}")
                    for h in range(2):
                        for m in range(4):
                            g = 4 * h + m
                            nc.vector.tensor_scalar_mul(
                                ys[:, 64 * g:64 * (g + 1)],
                                ut[:, 512 * h + 65 * m:512 * h + 65 * m + 64],
                                rec8[:, g:g + 1])
                    del u[i]
                    return ys

                def emit_transpose(i, ys):
                    # ys slices [128 tokens, 64 vchan] -> y2 [128 vchan, 512]
                    tp = rotp.tile([128, 1024], BF16, tag="rot", name=f"tp{i}")
                    for h in range(2):
                        for m in range(4):
                            nc.tensor.matmul(
                                tp[64 * h:64 * (h + 1), 128 * m:128 * (m + 1)],
                                ys[:, 64 * (4 * h + m):64 * (4 * h + m + 1)],
                                idt[:],
                                is_transpose=True,
                            )
                    y2 = ypool.tile([128, 512], BF16, tag="y", name=f"y{i}")
                    nc.vector.tensor_copy(y2[:], tp[:, 0:512])
                    return y2

                def emit_tail_epilogue(i):
                    # per-128-token-chunk chain: norm -> transpose -> y2 copy
                    # -> outproj -> stage -> DMA, so the stages pipeline
                    ut = u[i]
                    rec8 = sclp.tile([128, 8], F32, tag="rec")
                    for h in range(2):
                        nc.vector.reciprocal(
                            rec8[:, 4 * h:4 * (h + 1)].rearrange(
                                "p (m c) -> p m c", c=1),
                            ut[:, 512 * h:512 * h + 260].rearrange(
                                "p (m c) -> p m c", m=4)[:, :, 64:65])
                    ys = ypool.tile([128, 512], BF16, tag="ys", name=f"ys{i}")
                    tp = rotp.tile([128, 1024], BF16, tag="rot", name=f"tp{i}")
                    y2 = ypool.tile([128, 512], BF16, tag="y", name=f"y{i}")
                    stage = ypool.tile([128, 4096], BF16, tag="stage",
                                       name=f"st{i}")
                    for m in range(4):
                        for h in range(2):
                            g = 4 * h + m
                            nc.vector.tensor_scalar_mul(
                                ys[:, 64 * g:64 * (g + 1)],
                                ut[:, 512 * h + 65 * m:512 * h + 65 * m + 64],
                                rec8[:, g:g + 1])
                            nc.tensor.matmul(
                                tp[64 * h:64 * (h + 1), 128 * m:128 * (m + 1)],
                                ys[:, 64 * g:64 * (g + 1)],
                                idt[:],
                                is_transpose=True,
                            )
                        nc.scalar.copy(y2[:, 128 * m:128 * (m + 1)],
                                       tp[:, 128 * m:128 * (m + 1)])
                        ops_ = scp.tile([128, 1024], F32, tag="sc",
                                        name=f"opt{i}_{m}")
                        for n2 in range(2):
                            nc.tensor.matmul(
                                ops_[:, 512 * n2:512 * (n2 + 1)],
                                y2[:, 128 * m:128 * (m + 1)],
                                wos[:, 512 * n2:512 * (n2 + 1)],
                                start=True,
                                stop=True,
                            )
                        dst = stage[:, 1024 * m:1024 * (m + 1)]
                        if m % 2 == 0:
                            nc.vector.tensor_copy(dst, ops_[:])
                        else:
                            nc.scalar.copy(dst, ops_[:])
                        nc.sync.dma_start(
                            out[512 * i + 128 * m:512 * i + 128 * (m + 1), :],
                            dst,
                        )
                    del u[i]

                def emit_outproj(i, y, tail=False):
                    stage = ypool.tile([128, 4096], BF16, tag="stage", name=f"st{i}")
                    for m in range(4):
                        if tail:
                            # scores PSUM banks are free in the tail: pair the
                            # two 512-halves in one tile, one big stage copy
                            ops_ = scp.tile([128, 1024], F32, tag="sc",
                                            name=f"opt{i}_{m}")
                            for n2 in range(2):
                                nc.tensor.matmul(
                                    ops_[:, 512 * n2:512 * (n2 + 1)],
                                    y[:, 128 * m:128 * (m + 1)],
                                    wos[:, 512 * n2:512 * (n2 + 1)],
                                    start=True,
                                    stop=True,
                                )
                            dst = stage[:, 1024 * m:1024 * (m + 1)]
                            if m % 2 == 0:
                                nc.vector.tensor_copy(dst, ops_[:])
                            else:
                                nc.scalar.copy(dst, ops_[:])
                        else:
                            for n2 in range(2):
                                ops_ = rotp.tile([128, 512], F32, tag="rot")
                                nc.tensor.matmul(
                                    ops_[:],
                                    y[:, 128 * m:128 * (m + 1)],
                                    wos[:, 512 * n2:512 * (n2 + 1)],
                                    start=True,
                                    stop=True,
                                )
                                dst = stage[:, 1024 * m + 512 * n2:
                                            1024 * m + 512 * (n2 + 1)]
                                nc.vector.tensor_copy(dst, ops_[:])
                        # fire the DMA for this 128-token chunk immediately
                        nc.sync.dma_start(
                            out[512 * i + 128 * m:512 * i + 128 * (m + 1), :],
                            stage[:, 1024 * m:1024 * (m + 1)],
                        )

                # ---- schedule: A units paced under the B steps; AV lag
                # tapers from 8 (ACT queue deep early) to 1 (short tail)
                def av_lag(at):
                    return 10 if at < 44 else (4 if at < 56 else 1)

                extras = {
                    1: [("QKR", 1)], 2: [("V", 0, 0)], 3: [("V", 0, 1)],
                    4: [("QKR", 2)], 6: [("QKR", 3)], 8: [("V", 1, 0)],
                    10: [("V", 1, 1)], 12: [("V", 2, 0)], 14: [("V", 2, 1)],
                    16: [("V", 3, 0)], 18: [("V", 3, 1)],
                }
                emit_qkr(0)

                ys = {}
                av_q = []       # (i, j, idx when exp emitted)
                pending = []    # deferred events: (at_idx, kind, i)
                av_count = {}
                idx = 0
                for i in range(NT):
                    u[i] = upp.tile([128, 1024], F32, tag="u", name=f"u{i}")
                    nc.vector.memset(u[i][:, 0:260], 0.0)
                    nc.vector.memset(u[i][:, 512:772], 0.0)
                    av_count[i] = 0
                    for j in range(JT):
                        while av_q and av_q[0][2] + av_lag(idx) <= idx:
                            ai, aj, _ = av_q.pop(0)
                            emit_av(ai, aj, av_count[ai])
                            av_count[ai] += 1
                            if av_count[ai] == JT:
                                ys[ai] = emit_norm(ai)
                                pending.append((idx + 1, "tr", ai))
                        emit_scores(i, j)
                        av_q.append((i, j, idx))
                        for unit in extras.get(idx, ()):
                            if unit[0] == "QKR":
                                emit_qkr(unit[1])
                            else:
                                emit_v(unit[1], unit[2])
                        while pending and pending[0][0] <= idx:
                            _, kind, ei = pending.pop(0)
                            if kind == "tr":
                                ys[ei] = emit_transpose(ei, ys.pop(ei))
                                pending.append((idx + 1, "op", ei))
                            else:
                                emit_outproj(ei, ys.pop(ei))
                        idx += 1
                done_i = []
                for ai, aj, _ in av_q:
                    emit_av(ai, aj, av_count[ai])
                    av_count[ai] += 1
                    if av_count[ai] == JT:
                        done_i.append(ai)
                for _, kind, ei in sorted(pending):
                    if kind == "tr":
                        ys[ei] = emit_transpose(ei, ys.pop(ei))
                        emit_outproj(ei, ys.pop(ei), tail=True)
                    else:
                        emit_outproj(ei, ys.pop(ei), tail=True)
                for ai in done_i:
                    emit_tail_epilogue(ai)

            if loop_iters is None:
                emit_body()
            else:
                with tc.For_i(0, loop_iters, 1):
                    emit_body()

    nc.compile()
    return nc


_NC = None


def _get_nc():
    global _NC
    if _NC is None:
        _NC = build_program()
    return _NC


def make_in_maps(x, rope_emb_L_1_1_D, Wq, Wk, Wv, Wo):
    """Host-side prep: shard weights by head, transpose x, build rope tables.
    Everything shipped to the device as bf16."""
    import ml_dtypes

    bf16 = ml_dtypes.bfloat16
    x = np.asarray(x, dtype=np.float32)
    rope = np.asarray(rope_emb_L_1_1_D, dtype=np.float32).reshape(L, HD)
    Wq = np.asarray(Wq, dtype=np.float32)
    Wk = np.asarray(Wk, dtype=np.float32)
    Wv = np.asarray(Wv, dtype=np.float32)
    Wo = np.asarray(Wo, dtype=np.float32)

    xs_flat = x.reshape(L, D)  # B == 1
    # [128, (n k l)]: element (p, n, k, l) = x[512n+l, 128k+p]
    xT = np.ascontiguousarray(
        xs_flat.reshape(NT, 512, KT, 128).transpose(3, 0, 2, 1).reshape(
            128, NT * KT * 512)).astype(bf16)

    def wprep(wt):  # [D, C] -> [128, (k c)]: (p, k, c) = wt[128k+p, c]
        return np.ascontiguousarray(
            wt.reshape(KT, 128, C).transpose(1, 0, 2).reshape(128, KT * C)
        ).astype(bf16)

    cosT = np.cos(rope).T.astype(bf16)  # [HD, L]
    sinT = np.sin(rope).T.astype(bf16)

    # rot(q)[d'] = sum_k rmat[k, d'] q[k]; per 64-block: first 32 rows get
    # -q[d+32], last 32 get +q[d-32]  (signs folded in so sinT is plain sin)
    rmat = np.zeros((128, 128), dtype=np.float32)
    for b in (0, 64):
        for m in range(32):
            rmat[b + m + 32, b + m] = -1.0
        for m in range(32, 64):
            rmat[b + m - 32, b + m] = 1.0
    rmat = rmat.astype(bf16)

    scale = HD ** -0.5
    identm = np.eye(128, dtype=np.float32).astype(bf16)
    in_maps = []
    for c in range(NCORES):
        rows = slice(C * c, C * (c + 1))
        in_maps.append({
            "xT": xT,
            "wq": wprep((scale * Wq[rows, :]).T),
            "wk": wprep(Wk[rows, :].T),
            "wv": wprep(Wv[rows, :].T),
            "wo": np.ascontiguousarray(Wo[:, rows].T).astype(bf16),
            "cosT": cosT,
            "sinT": sinT,
            "rmat": rmat,
            "ident": identm,
        })
    return in_maps


class _Runner:
    """Persistent jitted SPMD executable (mirrors bass2jax.run_bass_via_pjrt
    but caches the compiled callable, and builds the donated output buffers
    on-device instead of shipping zeros through the tunnel)."""

    def __init__(self, nc):
        import jax
        import jax.numpy as jnp
        from jax.sharding import Mesh, PartitionSpec, NamedSharding
        from jax.experimental.shard_map import shard_map
        from concourse import bass2jax

        bass2jax.install_neuronx_cc_hook()
        self.jax = jax
        self.nc = nc
        part_name = nc.partition_id_tensor.name if nc.partition_id_tensor else None
        in_names, out_names, out_avals, zero_shapes = [], [], [], []
        for alloc in nc.m.functions[0].allocations:
            if not isinstance(alloc, mybir.MemoryLocationSet):
                continue
            name = alloc.memorylocations[0].name
            if alloc.kind == "ExternalInput":
                if name != part_name:
                    in_names.append(name)
            elif alloc.kind == "ExternalOutput":
                out_names.append(name)
                shape = tuple(alloc.tensor_shape)
                dtype = mybir.dt.np(alloc.dtype)
                out_avals.append(jax.core.ShapedArray(shape, dtype))
                zero_shapes.append((shape, dtype))
        self.in_names = list(in_names)
        self.out_names = list(out_names)
        self.out_avals = out_avals
        self.zero_shapes = zero_shapes
        n_params = len(in_names)
        n_outs = len(out_names)
        all_in_names = in_names + out_names
        if part_name is not None:
            all_in_names = all_in_names + [part_name]

        def _body(*args):
            operands = list(args)
            if part_name is not None:
                operands.append(bass2jax.partition_id_tensor())
            outs = bass2jax._bass_exec_p.bind(
                *operands,
                out_avals=tuple(out_avals),
                in_names=tuple(all_in_names),
                out_names=tuple(out_names),
                lowering_input_output_aliases=(),
                sim_require_finite=True,
                sim_require_nnan=True,
                nc=nc,
            )
            return tuple(outs)

        devices = jax.devices()[:NCORES]
        self.mesh = Mesh(np.asarray(devices), ("core",))
        self.pspec = PartitionSpec("core")
        self.sh = NamedSharding(self.mesh, self.pspec)
        in_specs = (self.pspec,) * (n_params + n_outs)
        out_specs = (self.pspec,) * n_outs
        self.sharded = jax.jit(
            shard_map(_body, mesh=self.mesh, in_specs=in_specs,
                      out_specs=out_specs, check_rep=False),
            donate_argnums=tuple(range(n_params, n_params + n_outs)),
            keep_unused=True,
        )
        # donated output buffers built on-device (fresh ones per call)
        self._zeros_fn = jax.jit(
            lambda: tuple(
                jnp.zeros((NCORES * s[0], *s[1:]), dt) for s, dt in zero_shapes
            ),
            out_shardings=tuple(self.sh for _ in zero_shapes),
        )

    def concat_inputs(self, in_maps):
        return [
            np.concatenate([np.asarray(m[name]) for m in in_maps], axis=0)
            for name in self.in_names
        ]

    def device_inputs(self, in_maps):
        return [self.jax.device_put(a, self.sh) for a in self.concat_inputs(in_maps)]

    def fresh_zeros(self):
        return list(self._zeros_fn())

    def __call__(self, dev_in, zeros):
        outs = self.sharded(*dev_in, *zeros)
        self.jax.block_until_ready(outs)
        return outs

    def run_np(self, in_maps):
        outs = self(self.device_inputs(in_maps), self.fresh_zeros())
        per_core = []
        for c in range(NCORES):
            d = {}
            for idx, name in enumerate(self.out_names):
                shape = self.out_avals[idx].shape
                d[name] = np.asarray(outs[idx]).reshape(NCORES, *shape)[c]
            per_core.append(d)
        return per_core


_RUNNER = None


def _get_runner():
    global _RUNNER
    if _RUNNER is None:
        _RUNNER = _Runner(_get_nc())
    return _RUNNER


def run(inputs):
    runner = _get_runner()
    in_maps = make_in_maps(**inputs)
    results = runner.run_np(in_maps)
    partial = np.zeros((L, D), dtype=np.float32)
    for r in results:
        partial += r["out"].astype(np.float32)
    return partial.reshape(T, H, W, B, D)


def kernel(**inputs):
    return run(inputs)

